# revision 28
# baseline (speedup 1.0000x reference)
"""AttentionGCNConv edge kernel for 8 Trainium2 NeuronCores (v2).

Strategy (edge-sharded SPMD, no cross-core communication):
  * Edges bucketed by destination-node range (ns nodes per core) so every
    gather is core-local, then sorted by col and packed into 8-lane SLOTS:
    all 8 lanes of a slot share one destination node, so one 72-byte
    indirect-DMA descriptor serves 8 edges.  The Q7 SWDGE fixed cost
    (994 ns/instruction) is amortized 8x vs. the per-edge baseline.
  * Node phase computes h = x@W+b, the exact per-scalar MLP f on h,
    gn = exp(f(h)), G = sum_c gn, and packs bf16 table rows
    {P_left = repeat2(h_lo)*gn [16], P_right = repeat2(h_hi) [16], G}.
  * Edge phase computes scores = edge_attr@W_edge+b_edge with BLOCK-DIAGONAL
    packed matmuls: 8 lanes x 9 contraction rows = 72-row lhsT so one matmul
    produces 1024 edge-scores (vs 128 in the naive per-chunk form).
    exp(f(score)) via a host-fitted product-of-quadratics polynomial
    (ACT Square + DVE fused ops), exact 16-branch fallback.
  * Combine: D = G + sum_c exp(f(score)), R = 1/D, out = {P_left*R,
    P_right*R*gea} per lane, written bf16 and widened to f32 on host.
"""
import numpy as np


# ---------------------------------------------------------------------------
# problem constants (hardcoded per the task statement)
# ---------------------------------------------------------------------------
N_NODES = 100000
E_EDGES = 1000000
IN_C = 64
C = 16          # OUT_C
ED = 8          # EDGE_D
NCORES = 8
P = 128
G_LANES = 8     # edges per lane-group (all share one destination node)
QG = 4          # lane-groups per quad-slot (4 consecutive nodes, one 288B desc)
TW = 36         # table row width in bf16 (72B): 16 P_left, 16 P_right, G, pad
KCQ = 4         # quad-columns processed per pipeline chunk (= 16 pseudo-cols)


class Cfg:
    def __init__(self, n_nodes, e_edges, ncores, nq_max, in_c=IN_C):
        self.ncores = ncores
        self.in_c = in_c
        # node shard: multiple of 128
        ns = -(-n_nodes // ncores)
        self.ns = -(-ns // P) * P
        self.nchunks = self.ns // P
        self.n_nodes = n_nodes
        self.e_edges = e_edges
        # quad grid: KQ columns of 128 quads, KQ multiple of KCQ
        kq = -(-nq_max // P)
        self.KQ = -(-kq // KCQ) * KCQ
        self.K = self.KQ * QG                 # pseudo-slot columns
        self.nslots = self.K * P
        self.lanes = self.nslots * G_LANES    # padded edge-lane count


# ---------------------------------------------------------------------------
# host-side derived parameters
# ---------------------------------------------------------------------------
def _f_scalar(s, w1, b1, w2, b2):
    z = s[..., None] * w1 + b1
    return (np.maximum(z, 0.0) * w2).sum(-1) + b2[0]


def fit_poly_factors(w1, b1, w2, b2, lo, hi):
    """Fit exp(f(s)) on [lo, hi] by a polynomial that factors into real
    quadratics (s+u)^2 + v scaled by alpha = c_lead^(1/nf).  Returns
    (factors, max_rel_err) or None if no degree works."""
    grid = np.linspace(lo, hi, 8192)
    target = np.exp(_f_scalar(grid, w1, b1, w2, b2))
    for deg in (6, 8, 10, 12, 14):
        ch = np.polynomial.chebyshev.Chebyshev.fit(grid, target, deg)
        p = ch.convert(kind=np.polynomial.Polynomial)
        c_lead = p.coef[-1]
        if c_lead <= 0:
            continue
        roots = p.roots()
        creal = sorted([r.real for r in roots if abs(r.imag) < 1e-12])
        ccplx = [r for r in roots if r.imag > 1e-12]
        if len(creal) % 2 != 0:
            continue
        quads = [(-r.real, r.imag ** 2) for r in ccplx]
        for i in range(0, len(creal), 2):
            r1, r2 = creal[i], creal[i + 1]
            quads.append((-(r1 + r2) / 2.0, -(((r1 - r2) / 2.0) ** 2)))
        nf = len(quads)
        alpha = c_lead ** (1.0 / nf)
        sa = float(np.sqrt(alpha))
        facs = [(sa, sa * u, alpha * v) for (u, v) in quads]
        acc = np.ones_like(grid)
        for (a, b, v) in facs:
            acc = acc * ((a * grid + b) ** 2 + v)
        rel = np.abs(acc - target) / np.abs(target)
        if rel.max() < 4.5e-3:
            return facs, float(rel.max())
    return None


def derive_params(inputs):
    w1 = np.asarray(inputs["w1"], np.float64)
    b1 = np.asarray(inputs["b1"], np.float64)
    w2 = np.asarray(inputs["w2"], np.float64)
    b2 = np.asarray(inputs["b2"], np.float64)
    W_edge = np.asarray(inputs["W_edge"], np.float64)
    b_edge = np.asarray(inputs["b_edge"], np.float64)

    aw = w1 * np.abs(w2)
    cw = b1 * np.abs(w2)
    sg = np.sign(w2)

    sigma_c = np.sqrt((W_edge ** 2).sum(0))
    lo = float((b_edge - 6.5 * sigma_c).min())
    hi = float((b_edge + 6.5 * sigma_c).max())
    fit = fit_poly_factors(w1, b1, w2, b2, lo, hi)

    # node-side poly over the EXACT h range (computed on host for the fit
    # range only; h itself is computed on device)
    x = np.asarray(inputs["x"], np.float64)
    W_lin = np.asarray(inputs["W_lin"], np.float64)
    b_lin = np.asarray(inputs["b_lin"], np.float64)
    h = x @ W_lin + b_lin
    mg = 1e-3 * (h.max() - h.min())
    fit_n = fit_poly_factors(w1, b1, w2, b2, float(h.min()) - mg,
                             float(h.max()) + mg)
    return {
        "aw": aw, "cw": cw, "sg": sg, "b2": float(b2[0]),
        "lo": lo, "hi": hi,
        "poly": None if fit is None else fit[0],
        "poly_err": None if fit is None else fit[1],
        "poly_n": None if fit_n is None else fit_n[0],
    }


# ---------------------------------------------------------------------------
# graph builder (SPMD, one graph for all cores)
# ---------------------------------------------------------------------------
def build_graph(cfg, dp):
    from concourse import bass, mybir
    import concourse.tile as tile

    f32 = mybir.dt.float32
    bf16 = mybir.dt.bfloat16
    i32 = mybir.dt.int32
    ALU = mybir.AluOpType
    ACTF = mybir.ActivationFunctionType

    nc = bass.Bass()
    xt = nc.declare_dram_parameter("xt", [cfg.in_c + 1, cfg.ns], bf16, isOutput=False)
    wlin = nc.declare_dram_parameter("wlin", [cfg.in_c + 1, C], bf16, isOutput=False)
    # packed edge attrs: [72, K, 128]: row 9l+d = attr d of lane l
    eat = nc.declare_dram_parameter("eat", [9 * G_LANES, cfg.K, P], bf16, isOutput=False)
    # block-diagonal W_edge: [72, 128]: rows 9l+d, cols 16l+c
    wbd = nc.declare_dram_parameter("wbd", [9 * G_LANES, P], bf16, isOutput=False)
    colv = nc.declare_dram_parameter("colv", [P, cfg.KQ], i32, isOutput=False)
    cvec = nc.declare_dram_parameter("cvec", [P, 32], f32, isOutput=False)
    out_e = nc.declare_dram_parameter(
        "out", [P, cfg.K * G_LANES * 2 * C], bf16, isOutput=True)
    # cvec columns: 0..15 = cw_k, 16 = b2, 17.. = poly bias b_i

    # Node-PAIR rows (72 bf16 = 144B) so the indirect-DMA descriptor length
    # (coef x 4B, an f32-element assumption in the DGE) covers exactly one
    # quad (288B) per offset.  +QG pad rows for quads based near ns.
    tableL = nc.dram_tensor("tableL", [(cfg.ns + 2 * QG) // 2, 2 * TW], bf16)

    NCH = cfg.nchunks
    NPOLY = 0 if dp["poly"] is None else len(dp["poly"])

    with tile.TileContext(nc) as tc, nc.allow_low_precision(
            reason="bf16 poly chain & recip; within the 2e-2 rel-err budget"):
        with tc.tile_pool(name="const", bufs=1) as constp:
            wlin_sb = constp.tile([cfg.in_c + 1, C], bf16)
            nc.sync.dma_start(out=wlin_sb[:], in_=wlin[:])
            wbd_sb = constp.tile([9 * G_LANES, P], bf16)
            nc.sync.dma_start(out=wbd_sb[:], in_=wbd[:])
            cvec_sb = constp.tile([P, 32], f32)
            nc.sync.dma_start(out=cvec_sb[:], in_=cvec[:])
            colv_sb = constp.tile([P, cfg.KQ], i32)
            nc.sync.dma_start(out=colv_sb[:], in_=colv[:])

            # ---------------- phase 0: node table ----------------
            with (
                tc.tile_pool(name="node_sb", bufs=1) as np_sb,
                tc.tile_pool(name="node_ps", bufs=2, space="PSUM") as np_ps,
            ):
                xt_sb = np_sb.tile([cfg.in_c + 1, cfg.ns], bf16)
                nc.sync.dma_start(out=xt_sb[:], in_=xt[:])

                h_wide = np_sb.tile([P, NCH * C], f32)
                for g0 in range(0, NCH, 32):
                    g1 = min(g0 + 32, NCH)
                    pst = np_ps.tile([P, 512], f32, tag="np_ps")
                    for j in range(g0, g1):
                        nc.tensor.matmul(
                            out=pst[:, (j - g0) * C:(j - g0 + 1) * C],
                            lhsT=xt_sb[:, j * P:(j + 1) * P],
                            rhs=wlin_sb[:],
                            start=True, stop=True,
                        )
                    nc.scalar.copy(
                        out=h_wide[:, g0 * C:g1 * C],
                        in_=pst[:, :(g1 - g0) * C],
                    )

                # gn = exp(f(h)) via the node-range poly (exact 16-branch
                # fallback if the fit failed)
                FW = NCH * C
                gn_wide = np_sb.tile([P, FW], f32)
                tmp = np_sb.tile([P, FW], f32)
                acc_a = np_sb.tile([P, FW], f32)
                acc_b = np_sb.tile([P, FW], f32)
                if dp["poly_n"] is not None:
                    npn = len(dp["poly_n"])
                    for i, (a, b, v) in enumerate(dp["poly_n"]):
                        nc.scalar.activation(
                            out=tmp[:], in_=h_wide[:], func=ACTF.Square,
                            bias=cvec_sb[:, 24 + i:25 + i], scale=float(a),
                        )
                        if i == 0:
                            nc.vector.tensor_scalar(
                                out=acc_a[:], in0=tmp[:], scalar1=float(v),
                                scalar2=None, op0=ALU.add,
                            )
                        else:
                            src, dst = (acc_a, acc_b) if i % 2 == 1 else (acc_b, acc_a)
                            nc.vector.scalar_tensor_tensor(
                                out=(gn_wide[:] if i == npn - 1 else dst[:]),
                                in0=tmp[:], scalar=float(v), in1=src[:],
                                op0=ALU.add, op1=ALU.mult,
                            )
                else:
                    for k in range(C):
                        nc.scalar.activation(
                            out=tmp[:], in_=h_wide[:], func=ACTF.Relu,
                            bias=cvec_sb[:, k:k + 1], scale=float(dp["aw"][k]),
                        )
                        src, dst = (acc_a, acc_b) if k % 2 == 1 else (acc_b, acc_a)
                        if k == 0:
                            nc.vector.tensor_scalar(
                                out=acc_a[:], in0=tmp[:],
                                scalar1=float(dp["sg"][k]), scalar2=None,
                                op0=ALU.mult,
                            )
                        else:
                            nc.vector.scalar_tensor_tensor(
                                out=dst[:], in0=tmp[:], scalar=float(dp["sg"][k]),
                                in1=src[:], op0=ALU.mult, op1=ALU.add,
                            )
                    acc_fin = acc_a if C % 2 == 1 else acc_b
                    nc.scalar.activation(
                        out=gn_wide[:], in_=acc_fin[:], func=ACTF.Exp,
                        bias=cvec_sb[:, 16:17], scale=1.0,
                    )
                g_wide = np_sb.tile([P, NCH], f32)
                nc.vector.tensor_reduce(
                    out=g_wide[:], in_=gn_wide[:].rearrange("p (j c) -> p j c", c=C),
                    axis=mybir.AxisListType.X, op=ALU.add,
                )

                # table rows (bf16, 72B): row n = j*128 + p
                rowbuf = np_sb.tile([P, NCH, TW], bf16)
                hv = h_wide[:].rearrange("p (j c) -> p j c", c=C)
                # P_left[c] = h[c//2] * gn[c]
                nc.vector.tensor_tensor(
                    out=rowbuf[:, :, 0:C].rearrange("p j (h two) -> p j h two", two=2),
                    in0=hv[:, :, 0:C // 2].unsqueeze(-1).to_broadcast(
                        [P, NCH, C // 2, 2]),
                    in1=gn_wide[:].rearrange(
                        "p (j h two) -> p j h two", h=C // 2, two=2),
                    op=ALU.mult,
                )
                # P_right[c] = h[8 + c//2]
                nc.vector.tensor_copy(
                    out=rowbuf[:, :, C:2 * C].rearrange(
                        "p j (h two) -> p j h two", two=2),
                    in_=hv[:, :, C // 2:C].unsqueeze(-1).to_broadcast(
                        [P, NCH, C // 2, 2]),
                )
                nc.vector.tensor_copy(
                    out=rowbuf[:, :, 2 * C:2 * C + 1],
                    in_=g_wide[:].unsqueeze(-1),
                )
                nc.vector.memset(rowbuf[:, :, 2 * C + 1:TW], 0.0)
                tl_rows = tableL[:].rearrange("r (h c) -> (r h) c", h=2)
                nc.sync.dma_start(
                    out=tl_rows[0:cfg.ns].rearrange("(j p) c -> p j c", p=P),
                    in_=rowbuf[:],
                )
                padrow = np_sb.tile([2 * QG, TW], bf16)
                nc.vector.memset(padrow[:], 1.0)
                nc.sync.dma_start(
                    out=tl_rows[cfg.ns:cfg.ns + 2 * QG], in_=padrow[:])

            # ----- fused edge pipeline: chunks of KCQ quad-columns -----
            NCHK = cfg.KQ // KCQ
            KC = KCQ * QG         # pseudo-slot columns per chunk
            FWC = KC * P          # lane-scores per chunk
            with (
                tc.tile_pool(name="ek", bufs=1) as ek,
                tc.tile_pool(name="esb", bufs=3) as esb,
                tc.tile_pool(name="eps", bufs=2, space="PSUM") as eps,
                tc.tile_pool(name="gsb", bufs=3) as gsb,
            ):
                gea_full = ek.tile([P, cfg.K * P], bf16)

                for t in range(NCHK):
                    kq0 = t * KCQ
                    k0 = t * KC
                    # gather first so the Q7 stays busy: one 288B descriptor
                    # per quad covers 4 consecutive table rows
                    grow = gsb.tile([P, KC, TW], bf16, tag="grow")
                    for kk in range(KCQ):
                        # dest MUST be a flat 2D AP: a 3D dest splits into
                        # one descriptor per middle-dim element, consuming
                        # extra (garbage) offsets
                        nc.gpsimd.indirect_dma_start(
                            out=grow[:, kk * QG:(kk + 1) * QG, :].rearrange(
                                "p q d -> p (q d)"),
                            out_offset=None,
                            in_=tableL[:],
                            in_offset=bass.IndirectOffsetOnAxis(
                                ap=colv_sb[:, kq0 + kk:kq0 + kk + 1], axis=0),
                        )

                    # packed ea matmuls: one per pseudo-column, into a
                    # 2-bank psum tile read directly by the poly ACTs
                    eat_t = esb.tile([9 * G_LANES, KC, P], bf16, tag="eat_t")
                    nc.sync.dma_start(out=eat_t[:], in_=eat[:, k0:k0 + KC, :])
                    pse = eps.tile([P, FWC], f32, tag="ea_ps")
                    for kk in range(KC):
                        nc.tensor.matmul(
                            out=pse[:, kk * P:(kk + 1) * P],
                            lhsT=eat_t[:, kk, :],
                            rhs=wbd_sb[:],
                            start=True, stop=True,
                        )

                    gea_t = gea_full[:, k0 * P:(k0 + KC) * P]
                    sq = esb.tile([P, FWC], bf16, tag="sq")
                    qa = esb.tile([P, FWC], bf16, tag="qa")
                    qb = esb.tile([P, FWC], bf16, tag="qb")
                    for i, (a, b, v) in enumerate(dp["poly"]):
                        nc.scalar.activation(
                            out=sq[:], in_=pse[:], func=ACTF.Square,
                            bias=cvec_sb[:, 17 + i:18 + i], scale=float(a),
                        )
                        if i == 0:
                            nc.vector.tensor_scalar(
                                out=qa[:], in0=sq[:], scalar1=float(v),
                                scalar2=None, op0=ALU.add,
                            )
                        else:
                            src, dst = (qa, qb) if i % 2 == 1 else (qb, qa)
                            last = i == NPOLY - 1
                            nc.vector.scalar_tensor_tensor(
                                out=(gea_t if last else dst[:]),
                                in0=sq[:], scalar=float(v), in1=src[:],
                                op0=ALU.add, op1=ALU.mult,
                            )

                    # easum per lane, D = G + easum, R = 1/D (bf16)
                    easum = gsb.tile([P, KC * G_LANES], f32, tag="easum")
                    nc.vector.tensor_reduce(
                        out=easum[:],
                        in_=gea_t.rearrange("p (k l c) -> p (k l) c", l=G_LANES, c=C),
                        axis=mybir.AxisListType.X, op=ALU.add,
                    )
                    d_t = gsb.tile([P, KC, G_LANES], f32, tag="d_t")
                    nc.vector.tensor_tensor(
                        out=d_t[:],
                        in0=grow[:, :, 2 * C:2 * C + 1].to_broadcast(
                            [P, KC, G_LANES]),
                        in1=easum[:].rearrange("p (k l) -> p k l", l=G_LANES),
                        op=ALU.add,
                    )
                    r_t = gsb.tile([P, KC, G_LANES], bf16, tag="r_t")
                    nc.vector.reciprocal(out=r_t[:], in_=d_t[:])

                    out_t = gsb.tile([P, KC, G_LANES, 2 * C], bf16, tag="out_t")
                    # left: P_left * R
                    nc.vector.tensor_tensor(
                        out=out_t[:, :, :, 0:C],
                        in0=grow[:, :, 0:C].unsqueeze(2).to_broadcast(
                            [P, KC, G_LANES, C]),
                        in1=r_t[:].unsqueeze(-1).to_broadcast(
                            [P, KC, G_LANES, C]),
                        op=ALU.mult,
                    )
                    # right: (gea * R) * P_right  (first mul on gpsimd)
                    wr = gsb.tile([P, KC, G_LANES, C], bf16, tag="wr")
                    nc.gpsimd.tensor_tensor(
                        out=wr[:],
                        in0=gea_t.rearrange("p (k l c) -> p k l c", l=G_LANES, c=C),
                        in1=r_t[:].unsqueeze(-1).to_broadcast(
                            [P, KC, G_LANES, C]),
                        op=ALU.mult,
                    )
                    nc.vector.tensor_tensor(
                        out=out_t[:, :, :, C:2 * C],
                        in0=wr[:],
                        in1=grow[:, :, C:2 * C].unsqueeze(2).to_broadcast(
                            [P, KC, G_LANES, C]),
                        op=ALU.mult,
                    )
                    nc.sync.dma_start(
                        out=out_e[:].rearrange(
                            "p (k l c) -> p k l c", l=G_LANES, c=2 * C)[
                            :, k0:k0 + KC, :, :],
                        in_=out_t[:],
                    )
    return nc


# ---------------------------------------------------------------------------
# walrus single-wait post-pass
# ---------------------------------------------------------------------------
def _split_multi_waits(nc):
    """This walrus build supports at most one sem-wait per instruction;
    hoist extra waits onto single-wait NoOps inserted just before."""
    from concourse import mybir
    ctr = [0]
    for f in nc.m.functions:
        for bb in f.blocks:
            il = bb.instructions
            new = []
            for inst in il:
                si = inst.sync_info
                waits = list(si.on_wait) if (si is not None and si.on_wait) else []
                if len(waits) > 1:
                    for w in waits[:-1]:
                        ctr[0] += 1
                        nop = mybir.InstNoOp(
                            name=f"splitw-{ctr[0]}", ins=[], outs=[])
                        nop.engine = inst.engine
                        nop.sync_info = mybir.SyncInfo(on_wait=[w], on_update=[])
                        new.append(nop)
                    si.on_wait = [waits[-1]]
                new.append(inst)
            il[:] = new
    return ctr[0]


def _patch_compiler_flags():
    """Enable the vector_dynamic_offsets DGE level (needed by the indirect
    gather); the default flag bundle disables it."""
    from concourse.compiler_utils import get_compiler_flags, set_compiler_flags
    flags = list(get_compiler_flags())
    if not flags:
        return
    out = []
    i = 0
    while i < len(flags):
        if flags[i] == "--internal-disable-dge-levels":
            i += 1
            while i < len(flags) and not flags[i].startswith("-"):
                i += 1
            continue
        out.append(flags[i])
        i += 1
    if "--internal-enable-dge-levels" in out:
        j = out.index("--internal-enable-dge-levels")
        if "vector_dynamic_offsets" not in out:
            out.insert(j + 1, "vector_dynamic_offsets")
    set_compiler_flags(out)


# ---------------------------------------------------------------------------
# host prep + entry
# ---------------------------------------------------------------------------
def _tobf16(x):
    import ml_dtypes
    return np.asarray(x, dtype=ml_dtypes.bfloat16)


def pack_core(col_loc, eidx, ns):
    """Quad-pack one core's edges: sort by local col; each node needs
    ceil(cnt/8) lane-groups; greedily cover lane-groups with quads of 4
    consecutive nodes (one 288B gather descriptor per quad).

    Returns (quad_base[int32 nq], lane_edge[int64 nq*4*8, global edge id
    or -1])."""
    order = np.argsort(col_loc, kind="stable")
    cs = col_loc[order]
    es = eidx[order]
    ne = len(cs)
    if ne == 0:
        return np.zeros(0, np.int32), np.zeros(0, np.int64)
    cnt = np.bincount(cs, minlength=ns)
    starts = np.concatenate([[0], np.cumsum(cnt)[:-1]])
    g = -(-cnt // G_LANES)
    rem = g.copy()
    nz = np.nonzero(rem)[0]
    quad_base = []
    consumed = []
    ptr = 0          # index into nz of first node with rem > 0
    n_nz = len(nz)
    while ptr < n_nz:
        n = nz[ptr]
        if rem[n] == 0:
            ptr += 1
            continue
        b = n & ~1          # even base: table rows are node pairs
        take = 0
        for j in range(QG):
            m = b + j
            if m < ns and rem[m] > 0:
                rem[m] -= 1
                take |= 1 << j
        quad_base.append(b)
        consumed.append(take)
    nq = len(quad_base)
    lane_edge = np.full((nq, QG, G_LANES), -1, np.int64)
    cursor = np.zeros(ns, np.int64)
    for q in range(nq):
        b = quad_base[q]
        tk = consumed[q]
        for j in range(QG):
            if tk & (1 << j):
                m = b + j
                s0 = starts[m] + cursor[m]
                k = min(G_LANES, cnt[m] - cursor[m])
                lane_edge[q, j, :k] = es[s0:s0 + k]
                cursor[m] += k
    return np.asarray(quad_base, np.int32), lane_edge.reshape(-1)


def host_prep(inputs, cfg, dp, packs):
    edge_attr = np.asarray(inputs["edge_attr"], np.float32)
    x = np.asarray(inputs["x"], np.float32)
    W_lin = np.asarray(inputs["W_lin"], np.float32)
    b_lin = np.asarray(inputs["b_lin"], np.float32)
    W_edge = np.asarray(inputs["W_edge"], np.float32)
    b_edge = np.asarray(inputs["b_edge"], np.float32)

    n = cfg.n_nodes
    nt_all = cfg.ns * cfg.ncores
    xt_all = np.zeros((cfg.in_c + 1, nt_all), np.float32)
    xt_all[:cfg.in_c, :n] = x.T
    xt_all[cfg.in_c, :] = 1.0
    xt_all = _tobf16(xt_all)
    wlin_aug = _tobf16(np.concatenate([W_lin, b_lin[None, :]], 0))

    # block-diagonal W_edge [72, 128]
    wbd = np.zeros((9 * G_LANES, P), np.float32)
    for l in range(G_LANES):
        wbd[9 * l:9 * l + ED, C * l:C * (l + 1)] = W_edge
        wbd[9 * l + ED, C * l:C * (l + 1)] = b_edge
    wbd = _tobf16(wbd)

    cv = np.zeros(32, np.float32)
    cv[:C] = dp["cw"]
    cv[16] = dp["b2"]
    for i, (_a, b, _v) in enumerate(dp["poly"]):
        cv[17 + i] = b
    if dp["poly_n"] is not None:
        assert len(dp["poly_n"]) <= 8
        for i, (_a, b, _v) in enumerate(dp["poly_n"]):
            cv[24 + i] = b
    cvec_arr = np.broadcast_to(cv, (P, 32)).copy()

    in_maps = []
    for c in range(cfg.ncores):
        quad_base, lane_edge = packs[c]
        nq = len(quad_base)
        # quad q -> (p = q % 128, kq = q // 128); offsets are PAIR-row ids
        colw = np.zeros((P, cfg.KQ), np.int32)
        ql = np.arange(nq)
        colw[ql % P, ql // P] = quad_base // 2
        # eat packed [72, K, 128]: pseudo-slot (p, k=kq*QG+j) lane l
        eatp = np.zeros((9 * G_LANES, cfg.K, P), np.float32)
        le = lane_edge.reshape(nq, QG, G_LANES)
        valid = le >= 0
        ea_l = np.zeros((nq, QG, G_LANES, ED), np.float32)
        ea_l[valid] = edge_attr[le[valid]]
        pp = (ql % P)
        kq = (ql // P)
        for j in range(QG):
            kk = kq * QG + j
            for l in range(G_LANES):
                eatp[9 * l:9 * l + ED, kk, pp] = ea_l[:, j, l, :].T
                eatp[9 * l + ED, kk, pp] = valid[:, j, l].astype(np.float32)
        in_maps.append({
            "xt": np.ascontiguousarray(xt_all[:, c * cfg.ns:(c + 1) * cfg.ns]),
            "wlin": wlin_aug,
            "eat": _tobf16(eatp),
            "wbd": wbd,
            "colv": colw,
            "cvec": cvec_arr,
        })
    return in_maps


def run(inputs, trace=False):
    from concourse.bass_utils import run_bass_kernel_spmd

    _patch_compiler_flags()
    col = np.asarray(inputs["col"], np.int32)
    n_nodes = inputs["x"].shape[0]
    e_edges = col.shape[0]

    ns = -(-(-(-n_nodes // NCORES)) // P) * P
    owner = np.minimum(col // ns, NCORES - 1)
    packs = []
    nq_max = 1
    for c in range(NCORES):
        eidx = np.nonzero(owner == c)[0]
        qb, le = pack_core(col[eidx] - c * ns, eidx, ns)
        packs.append((qb, le))
        nq_max = max(nq_max, len(qb))
    cfg = Cfg(n_nodes, e_edges, NCORES, nq_max)

    dp = derive_params(inputs)
    assert dp["poly"] is not None, "poly fit failed"
    nc = build_graph(cfg, dp)
    _split_multi_waits(nc)
    in_maps = host_prep(inputs, cfg, dp, packs)
    res = run_bass_kernel_spmd(nc, in_maps, list(range(cfg.ncores)), trace=trace)
    full = np.empty((e_edges, 2 * C), np.float32)
    for c in range(cfg.ncores):
        quad_base, lane_edge = packs[c]
        o = np.asarray(res.results[c]["out"]).astype(np.float32)
        # [P, KQ, QG, G_LANES, 2C] -> quad q=(p + 128kq), group j, lane l
        o = o.reshape(P, cfg.KQ, QG, G_LANES, 2 * C).transpose(1, 0, 2, 3, 4)
        o = o.reshape(cfg.KQ * P * QG * G_LANES, 2 * C)
        valid = lane_edge >= 0
        full[lane_edge[valid]] = o[:len(lane_edge)][valid]
    return full, res


def kernel(**inputs):
    full, _ = run(inputs, trace=False)
    return full


# revision 29
# speedup vs baseline: 1.0714x; 1.0714x over previous
"""AttentionGCNConv edge kernel for 8 Trainium2 NeuronCores (v2).

Strategy (edge-sharded SPMD, no cross-core communication):
  * Edges bucketed by destination-node range (ns nodes per core) so every
    gather is core-local, then sorted by col and packed into 8-lane SLOTS:
    all 8 lanes of a slot share one destination node, so one 72-byte
    indirect-DMA descriptor serves 8 edges.  The Q7 SWDGE fixed cost
    (994 ns/instruction) is amortized 8x vs. the per-edge baseline.
  * Node phase computes h = x@W+b, the exact per-scalar MLP f on h,
    gn = exp(f(h)), G = sum_c gn, and packs bf16 table rows
    {P_left = repeat2(h_lo)*gn [16], P_right = repeat2(h_hi) [16], G}.
  * Edge phase computes scores = edge_attr@W_edge+b_edge with BLOCK-DIAGONAL
    packed matmuls: 8 lanes x 9 contraction rows = 72-row lhsT so one matmul
    produces 1024 edge-scores (vs 128 in the naive per-chunk form).
    exp(f(score)) via a host-fitted product-of-quadratics polynomial
    (ACT Square + DVE fused ops), exact 16-branch fallback.
  * Combine: D = G + sum_c exp(f(score)), R = 1/D, out = {P_left*R,
    P_right*R*gea} per lane, written bf16 and widened to f32 on host.
"""
import numpy as np


# ---------------------------------------------------------------------------
# problem constants (hardcoded per the task statement)
# ---------------------------------------------------------------------------
N_NODES = 100000
E_EDGES = 1000000
IN_C = 64
C = 16          # OUT_C
ED = 8          # EDGE_D
NCORES = 8
P = 128
G_LANES = 8     # edges per lane-group (all share one destination node)
QG = 2          # lane-groups per pair-slot (2 consecutive nodes, one 144B desc)
TW = 36         # table row width in bf16 (72B): 16 P_left, 16 P_right, G, pad
KCQ = 8         # pair-columns processed per pipeline chunk (= 16 pseudo-cols)


class Cfg:
    def __init__(self, n_nodes, e_edges, ncores, nq_max, in_c=IN_C):
        self.ncores = ncores
        self.in_c = in_c
        # node shard: multiple of 128
        ns = -(-n_nodes // ncores)
        self.ns = -(-ns // P) * P
        self.nchunks = self.ns // P
        self.n_nodes = n_nodes
        self.e_edges = e_edges
        # quad grid: KQ columns of 128 quads, KQ multiple of KCQ
        kq = -(-nq_max // P)
        self.KQ = -(-kq // KCQ) * KCQ
        self.K = self.KQ * QG                 # pseudo-slot columns
        self.nslots = self.K * P
        self.lanes = self.nslots * G_LANES    # padded edge-lane count


# ---------------------------------------------------------------------------
# host-side derived parameters
# ---------------------------------------------------------------------------
def _f_scalar(s, w1, b1, w2, b2):
    z = s[..., None] * w1 + b1
    return (np.maximum(z, 0.0) * w2).sum(-1) + b2[0]


def fit_poly_factors(w1, b1, w2, b2, lo, hi):
    """Fit exp(f(s)) on [lo, hi] by a polynomial that factors into real
    quadratics (s+u)^2 + v scaled by alpha = c_lead^(1/nf).  Returns
    (factors, max_rel_err) or None if no degree works."""
    grid = np.linspace(lo, hi, 8192)
    target = np.exp(_f_scalar(grid, w1, b1, w2, b2))
    for deg in (6, 8, 10, 12, 14):
        ch = np.polynomial.chebyshev.Chebyshev.fit(grid, target, deg)
        p = ch.convert(kind=np.polynomial.Polynomial)
        c_lead = p.coef[-1]
        if c_lead <= 0:
            continue
        roots = p.roots()
        creal = sorted([r.real for r in roots if abs(r.imag) < 1e-12])
        ccplx = [r for r in roots if r.imag > 1e-12]
        if len(creal) % 2 != 0:
            continue
        quads = [(-r.real, r.imag ** 2) for r in ccplx]
        for i in range(0, len(creal), 2):
            r1, r2 = creal[i], creal[i + 1]
            quads.append((-(r1 + r2) / 2.0, -(((r1 - r2) / 2.0) ** 2)))
        nf = len(quads)
        alpha = c_lead ** (1.0 / nf)
        sa = float(np.sqrt(alpha))
        facs = [(sa, sa * u, alpha * v) for (u, v) in quads]
        acc = np.ones_like(grid)
        for (a, b, v) in facs:
            acc = acc * ((a * grid + b) ** 2 + v)
        rel = np.abs(acc - target) / np.abs(target)
        if rel.max() < 4.5e-3:
            return facs, float(rel.max())
    return None


def derive_params(inputs):
    w1 = np.asarray(inputs["w1"], np.float64)
    b1 = np.asarray(inputs["b1"], np.float64)
    w2 = np.asarray(inputs["w2"], np.float64)
    b2 = np.asarray(inputs["b2"], np.float64)
    W_edge = np.asarray(inputs["W_edge"], np.float64)
    b_edge = np.asarray(inputs["b_edge"], np.float64)

    aw = w1 * np.abs(w2)
    cw = b1 * np.abs(w2)
    sg = np.sign(w2)

    sigma_c = np.sqrt((W_edge ** 2).sum(0))
    lo = float((b_edge - 6.5 * sigma_c).min())
    hi = float((b_edge + 6.5 * sigma_c).max())
    fit = fit_poly_factors(w1, b1, w2, b2, lo, hi)

    # node-side poly over the EXACT h range (computed on host for the fit
    # range only; h itself is computed on device)
    x = np.asarray(inputs["x"], np.float64)
    W_lin = np.asarray(inputs["W_lin"], np.float64)
    b_lin = np.asarray(inputs["b_lin"], np.float64)
    h = x @ W_lin + b_lin
    mg = 1e-3 * (h.max() - h.min())
    fit_n = fit_poly_factors(w1, b1, w2, b2, float(h.min()) - mg,
                             float(h.max()) + mg)
    return {
        "aw": aw, "cw": cw, "sg": sg, "b2": float(b2[0]),
        "lo": lo, "hi": hi,
        "poly": None if fit is None else fit[0],
        "poly_err": None if fit is None else fit[1],
        "poly_n": None if fit_n is None else fit_n[0],
    }


# ---------------------------------------------------------------------------
# graph builder (SPMD, one graph for all cores)
# ---------------------------------------------------------------------------
def build_graph(cfg, dp):
    from concourse import bass, mybir
    import concourse.tile as tile

    f32 = mybir.dt.float32
    bf16 = mybir.dt.bfloat16
    i32 = mybir.dt.int32
    ALU = mybir.AluOpType
    ACTF = mybir.ActivationFunctionType

    nc = bass.Bass()
    xt = nc.declare_dram_parameter("xt", [cfg.in_c + 1, cfg.ns], bf16, isOutput=False)
    wlin = nc.declare_dram_parameter("wlin", [cfg.in_c + 1, C], bf16, isOutput=False)
    # packed edge attrs: [72, K, 128]: row 9l+d = attr d of lane l
    eat = nc.declare_dram_parameter("eat", [9 * G_LANES, cfg.K, P], bf16, isOutput=False)
    # block-diagonal W_edge: [72, 128]: rows 9l+d, cols 16l+c
    wbd = nc.declare_dram_parameter("wbd", [9 * G_LANES, P], bf16, isOutput=False)
    colv = nc.declare_dram_parameter("colv", [P, cfg.KQ], i32, isOutput=False)
    cvec = nc.declare_dram_parameter("cvec", [P, 32], f32, isOutput=False)
    out_e = nc.declare_dram_parameter(
        "out", [P, cfg.K * G_LANES * 2 * C], bf16, isOutput=True)
    # cvec columns: 0..15 = cw_k, 16 = b2, 17.. = poly bias b_i

    # Node-PAIR rows (72 bf16 = 144B) so the indirect-DMA descriptor length
    # (coef x 4B, an f32-element assumption in the DGE) covers exactly one
    # quad (288B) per offset.  +QG pad rows for quads based near ns.
    tableL = nc.dram_tensor("tableL", [(cfg.ns + 2 * QG) // 2, 2 * TW], bf16)

    NCH = cfg.nchunks
    NPOLY = 0 if dp["poly"] is None else len(dp["poly"])

    with tile.TileContext(nc) as tc, nc.allow_low_precision(
            reason="bf16 poly chain & recip; within the 2e-2 rel-err budget"):
        with tc.tile_pool(name="const", bufs=1) as constp:
            wlin_sb = constp.tile([cfg.in_c + 1, C], bf16)
            nc.sync.dma_start(out=wlin_sb[:], in_=wlin[:])
            wbd_sb = constp.tile([9 * G_LANES, P], bf16)
            nc.sync.dma_start(out=wbd_sb[:], in_=wbd[:])
            cvec_sb = constp.tile([P, 32], f32)
            nc.sync.dma_start(out=cvec_sb[:], in_=cvec[:])
            colv_sb = constp.tile([P, cfg.KQ], i32)
            nc.sync.dma_start(out=colv_sb[:], in_=colv[:])

            # ---------------- phase 0: node table ----------------
            with (
                tc.tile_pool(name="node_sb", bufs=1) as np_sb,
                tc.tile_pool(name="node_ps", bufs=2, space="PSUM") as np_ps,
            ):
                xt_sb = np_sb.tile([cfg.in_c + 1, cfg.ns], bf16)
                nc.sync.dma_start(out=xt_sb[:], in_=xt[:])

                h_wide = np_sb.tile([P, NCH * C], f32)
                for g0 in range(0, NCH, 32):
                    g1 = min(g0 + 32, NCH)
                    pst = np_ps.tile([P, 512], f32, tag="np_ps")
                    for j in range(g0, g1):
                        nc.tensor.matmul(
                            out=pst[:, (j - g0) * C:(j - g0 + 1) * C],
                            lhsT=xt_sb[:, j * P:(j + 1) * P],
                            rhs=wlin_sb[:],
                            start=True, stop=True,
                        )
                    nc.scalar.copy(
                        out=h_wide[:, g0 * C:g1 * C],
                        in_=pst[:, :(g1 - g0) * C],
                    )

                # gn = exp(f(h)) via the node-range poly (exact 16-branch
                # fallback if the fit failed)
                FW = NCH * C
                gn_wide = np_sb.tile([P, FW], f32)
                tmp = np_sb.tile([P, FW], f32)
                acc_a = np_sb.tile([P, FW], f32)
                acc_b = np_sb.tile([P, FW], f32)
                if dp["poly_n"] is not None:
                    npn = len(dp["poly_n"])
                    for i, (a, b, v) in enumerate(dp["poly_n"]):
                        nc.scalar.activation(
                            out=tmp[:], in_=h_wide[:], func=ACTF.Square,
                            bias=cvec_sb[:, 24 + i:25 + i], scale=float(a),
                        )
                        if i == 0:
                            nc.vector.tensor_scalar(
                                out=acc_a[:], in0=tmp[:], scalar1=float(v),
                                scalar2=None, op0=ALU.add,
                            )
                        else:
                            src, dst = (acc_a, acc_b) if i % 2 == 1 else (acc_b, acc_a)
                            nc.vector.scalar_tensor_tensor(
                                out=(gn_wide[:] if i == npn - 1 else dst[:]),
                                in0=tmp[:], scalar=float(v), in1=src[:],
                                op0=ALU.add, op1=ALU.mult,
                            )
                else:
                    for k in range(C):
                        nc.scalar.activation(
                            out=tmp[:], in_=h_wide[:], func=ACTF.Relu,
                            bias=cvec_sb[:, k:k + 1], scale=float(dp["aw"][k]),
                        )
                        src, dst = (acc_a, acc_b) if k % 2 == 1 else (acc_b, acc_a)
                        if k == 0:
                            nc.vector.tensor_scalar(
                                out=acc_a[:], in0=tmp[:],
                                scalar1=float(dp["sg"][k]), scalar2=None,
                                op0=ALU.mult,
                            )
                        else:
                            nc.vector.scalar_tensor_tensor(
                                out=dst[:], in0=tmp[:], scalar=float(dp["sg"][k]),
                                in1=src[:], op0=ALU.mult, op1=ALU.add,
                            )
                    acc_fin = acc_a if C % 2 == 1 else acc_b
                    nc.scalar.activation(
                        out=gn_wide[:], in_=acc_fin[:], func=ACTF.Exp,
                        bias=cvec_sb[:, 16:17], scale=1.0,
                    )
                g_wide = np_sb.tile([P, NCH], f32)
                nc.vector.tensor_reduce(
                    out=g_wide[:], in_=gn_wide[:].rearrange("p (j c) -> p j c", c=C),
                    axis=mybir.AxisListType.X, op=ALU.add,
                )

                # table rows (bf16, 72B): row n = j*128 + p
                rowbuf = np_sb.tile([P, NCH, TW], bf16)
                hv = h_wide[:].rearrange("p (j c) -> p j c", c=C)
                # P_left[c] = h[c//2] * gn[c]
                nc.vector.tensor_tensor(
                    out=rowbuf[:, :, 0:C].rearrange("p j (h two) -> p j h two", two=2),
                    in0=hv[:, :, 0:C // 2].unsqueeze(-1).to_broadcast(
                        [P, NCH, C // 2, 2]),
                    in1=gn_wide[:].rearrange(
                        "p (j h two) -> p j h two", h=C // 2, two=2),
                    op=ALU.mult,
                )
                # P_right[c] = h[8 + c//2]
                nc.vector.tensor_copy(
                    out=rowbuf[:, :, C:2 * C].rearrange(
                        "p j (h two) -> p j h two", two=2),
                    in_=hv[:, :, C // 2:C].unsqueeze(-1).to_broadcast(
                        [P, NCH, C // 2, 2]),
                )
                nc.vector.tensor_copy(
                    out=rowbuf[:, :, 2 * C:2 * C + 1],
                    in_=g_wide[:].unsqueeze(-1),
                )
                nc.vector.memset(rowbuf[:, :, 2 * C + 1:TW], 0.0)
                tl_rows = tableL[:].rearrange("r (h c) -> (r h) c", h=2)
                nc.sync.dma_start(
                    out=tl_rows[0:cfg.ns].rearrange("(j p) c -> p j c", p=P),
                    in_=rowbuf[:],
                )
                padrow = np_sb.tile([2 * QG, TW], bf16)
                nc.vector.memset(padrow[:], 1.0)
                nc.sync.dma_start(
                    out=tl_rows[cfg.ns:cfg.ns + 2 * QG], in_=padrow[:])

            # ----- fused edge pipeline: chunks of KCQ quad-columns -----
            NCHK = cfg.KQ // KCQ
            KC = KCQ * QG         # pseudo-slot columns per chunk
            FWC = KC * P          # lane-scores per chunk
            with (
                tc.tile_pool(name="ek", bufs=1) as ek,
                tc.tile_pool(name="esb", bufs=3) as esb,
                tc.tile_pool(name="eps", bufs=2, space="PSUM") as eps,
                tc.tile_pool(name="gsb", bufs=3) as gsb,
            ):
                gea_full = ek.tile([P, cfg.K * P], bf16)

                for t in range(NCHK):
                    kq0 = t * KCQ
                    k0 = t * KC
                    # gather first so the Q7 stays busy: one 288B descriptor
                    # per quad covers 4 consecutive table rows
                    grow = gsb.tile([P, KC, TW], bf16, tag="grow")
                    for kk in range(KCQ):
                        # dest MUST be a flat 2D AP: a 3D dest splits into
                        # one descriptor per middle-dim element, consuming
                        # extra (garbage) offsets
                        nc.gpsimd.indirect_dma_start(
                            out=grow[:, kk * QG:(kk + 1) * QG, :].rearrange(
                                "p q d -> p (q d)"),
                            out_offset=None,
                            in_=tableL[:],
                            in_offset=bass.IndirectOffsetOnAxis(
                                ap=colv_sb[:, kq0 + kk:kq0 + kk + 1], axis=0),
                        )

                    # packed ea matmuls: one per pseudo-column, into a
                    # 2-bank psum tile read directly by the poly ACTs
                    eat_t = esb.tile([9 * G_LANES, KC, P], bf16, tag="eat_t")
                    nc.sync.dma_start(out=eat_t[:], in_=eat[:, k0:k0 + KC, :])
                    pse = eps.tile([P, FWC], f32, tag="ea_ps")
                    for kk in range(KC):
                        nc.tensor.matmul(
                            out=pse[:, kk * P:(kk + 1) * P],
                            lhsT=eat_t[:, kk, :],
                            rhs=wbd_sb[:],
                            start=True, stop=True,
                        )

                    gea_t = gea_full[:, k0 * P:(k0 + KC) * P]
                    sq = esb.tile([P, FWC], bf16, tag="sq")
                    qa = esb.tile([P, FWC], bf16, tag="qa")
                    qb = esb.tile([P, FWC], bf16, tag="qb")
                    for i, (a, b, v) in enumerate(dp["poly"]):
                        nc.scalar.activation(
                            out=sq[:], in_=pse[:], func=ACTF.Square,
                            bias=cvec_sb[:, 17 + i:18 + i], scale=float(a),
                        )
                        if i == 0:
                            nc.vector.tensor_scalar(
                                out=qa[:], in0=sq[:], scalar1=float(v),
                                scalar2=None, op0=ALU.add,
                            )
                        else:
                            src, dst = (qa, qb) if i % 2 == 1 else (qb, qa)
                            last = i == NPOLY - 1
                            nc.vector.scalar_tensor_tensor(
                                out=(gea_t if last else dst[:]),
                                in0=sq[:], scalar=float(v), in1=src[:],
                                op0=ALU.add, op1=ALU.mult,
                            )

                    # easum per lane, D = G + easum, R = 1/D (bf16)
                    easum = gsb.tile([P, KC * G_LANES], f32, tag="easum")
                    nc.vector.tensor_reduce(
                        out=easum[:],
                        in_=gea_t.rearrange("p (k l c) -> p (k l) c", l=G_LANES, c=C),
                        axis=mybir.AxisListType.X, op=ALU.add,
                    )
                    d_t = gsb.tile([P, KC, G_LANES], f32, tag="d_t")
                    nc.vector.tensor_tensor(
                        out=d_t[:],
                        in0=grow[:, :, 2 * C:2 * C + 1].to_broadcast(
                            [P, KC, G_LANES]),
                        in1=easum[:].rearrange("p (k l) -> p k l", l=G_LANES),
                        op=ALU.add,
                    )
                    r_t = gsb.tile([P, KC, G_LANES], bf16, tag="r_t")
                    nc.vector.reciprocal(out=r_t[:], in_=d_t[:])

                    out_t = gsb.tile([P, KC, G_LANES, 2 * C], bf16, tag="out_t")
                    # left: P_left * R
                    nc.vector.tensor_tensor(
                        out=out_t[:, :, :, 0:C],
                        in0=grow[:, :, 0:C].unsqueeze(2).to_broadcast(
                            [P, KC, G_LANES, C]),
                        in1=r_t[:].unsqueeze(-1).to_broadcast(
                            [P, KC, G_LANES, C]),
                        op=ALU.mult,
                    )
                    # right: (gea * R) * P_right
                    wr = gsb.tile([P, KC, G_LANES, C], bf16, tag="wr")
                    nc.vector.tensor_tensor(
                        out=wr[:],
                        in0=gea_t.rearrange("p (k l c) -> p k l c", l=G_LANES, c=C),
                        in1=r_t[:].unsqueeze(-1).to_broadcast(
                            [P, KC, G_LANES, C]),
                        op=ALU.mult,
                    )
                    nc.vector.tensor_tensor(
                        out=out_t[:, :, :, C:2 * C],
                        in0=wr[:],
                        in1=grow[:, :, C:2 * C].unsqueeze(2).to_broadcast(
                            [P, KC, G_LANES, C]),
                        op=ALU.mult,
                    )
                    nc.sync.dma_start(
                        out=out_e[:].rearrange(
                            "p (k l c) -> p k l c", l=G_LANES, c=2 * C)[
                            :, k0:k0 + KC, :, :],
                        in_=out_t[:],
                    )
    return nc


# ---------------------------------------------------------------------------
# walrus single-wait post-pass
# ---------------------------------------------------------------------------
def _split_multi_waits(nc):
    """This walrus build supports at most one sem-wait per instruction;
    hoist extra waits onto single-wait NoOps inserted just before."""
    from concourse import mybir
    ctr = [0]
    for f in nc.m.functions:
        for bb in f.blocks:
            il = bb.instructions
            new = []
            for inst in il:
                si = inst.sync_info
                waits = list(si.on_wait) if (si is not None and si.on_wait) else []
                if len(waits) > 1:
                    for w in waits[:-1]:
                        ctr[0] += 1
                        nop = mybir.InstNoOp(
                            name=f"splitw-{ctr[0]}", ins=[], outs=[])
                        nop.engine = inst.engine
                        nop.sync_info = mybir.SyncInfo(on_wait=[w], on_update=[])
                        new.append(nop)
                    si.on_wait = [waits[-1]]
                new.append(inst)
            il[:] = new
    return ctr[0]


def _patch_compiler_flags():
    """Enable the vector_dynamic_offsets DGE level (needed by the indirect
    gather); the default flag bundle disables it."""
    from concourse.compiler_utils import get_compiler_flags, set_compiler_flags
    flags = list(get_compiler_flags())
    if not flags:
        return
    out = []
    i = 0
    while i < len(flags):
        if flags[i] == "--internal-disable-dge-levels":
            i += 1
            while i < len(flags) and not flags[i].startswith("-"):
                i += 1
            continue
        out.append(flags[i])
        i += 1
    if "--internal-enable-dge-levels" in out:
        j = out.index("--internal-enable-dge-levels")
        if "vector_dynamic_offsets" not in out:
            out.insert(j + 1, "vector_dynamic_offsets")
    set_compiler_flags(out)


# ---------------------------------------------------------------------------
# host prep + entry
# ---------------------------------------------------------------------------
def _tobf16(x):
    import ml_dtypes
    return np.asarray(x, dtype=ml_dtypes.bfloat16)


def pack_core(col_loc, eidx, ns):
    """Quad-pack one core's edges: sort by local col; each node needs
    ceil(cnt/8) lane-groups; greedily cover lane-groups with quads of 4
    consecutive nodes (one 288B gather descriptor per quad).

    Returns (quad_base[int32 nq], lane_edge[int64 nq*4*8, global edge id
    or -1])."""
    order = np.argsort(col_loc, kind="stable")
    cs = col_loc[order]
    es = eidx[order]
    ne = len(cs)
    if ne == 0:
        return np.zeros(0, np.int32), np.zeros(0, np.int64)
    cnt = np.bincount(cs, minlength=ns)
    starts = np.concatenate([[0], np.cumsum(cnt)[:-1]])
    g = -(-cnt // G_LANES)
    rem = g.copy()
    nz = np.nonzero(rem)[0]
    quad_base = []
    consumed = []
    ptr = 0          # index into nz of first node with rem > 0
    n_nz = len(nz)
    while ptr < n_nz:
        n = nz[ptr]
        if rem[n] == 0:
            ptr += 1
            continue
        b = n & ~1          # even base: table rows are node pairs
        take = 0
        for j in range(QG):
            m = b + j
            if m < ns and rem[m] > 0:
                rem[m] -= 1
                take |= 1 << j
        quad_base.append(b)
        consumed.append(take)
    nq = len(quad_base)
    lane_edge = np.full((nq, QG, G_LANES), -1, np.int64)
    cursor = np.zeros(ns, np.int64)
    for q in range(nq):
        b = quad_base[q]
        tk = consumed[q]
        for j in range(QG):
            if tk & (1 << j):
                m = b + j
                s0 = starts[m] + cursor[m]
                k = min(G_LANES, cnt[m] - cursor[m])
                lane_edge[q, j, :k] = es[s0:s0 + k]
                cursor[m] += k
    return np.asarray(quad_base, np.int32), lane_edge.reshape(-1)


def host_prep(inputs, cfg, dp, packs):
    edge_attr = np.asarray(inputs["edge_attr"], np.float32)
    x = np.asarray(inputs["x"], np.float32)
    W_lin = np.asarray(inputs["W_lin"], np.float32)
    b_lin = np.asarray(inputs["b_lin"], np.float32)
    W_edge = np.asarray(inputs["W_edge"], np.float32)
    b_edge = np.asarray(inputs["b_edge"], np.float32)

    n = cfg.n_nodes
    nt_all = cfg.ns * cfg.ncores
    xt_all = np.zeros((cfg.in_c + 1, nt_all), np.float32)
    xt_all[:cfg.in_c, :n] = x.T
    xt_all[cfg.in_c, :] = 1.0
    xt_all = _tobf16(xt_all)
    wlin_aug = _tobf16(np.concatenate([W_lin, b_lin[None, :]], 0))

    # block-diagonal W_edge [72, 128]
    wbd = np.zeros((9 * G_LANES, P), np.float32)
    for l in range(G_LANES):
        wbd[9 * l:9 * l + ED, C * l:C * (l + 1)] = W_edge
        wbd[9 * l + ED, C * l:C * (l + 1)] = b_edge
    wbd = _tobf16(wbd)

    cv = np.zeros(32, np.float32)
    cv[:C] = dp["cw"]
    cv[16] = dp["b2"]
    for i, (_a, b, _v) in enumerate(dp["poly"]):
        cv[17 + i] = b
    if dp["poly_n"] is not None:
        assert len(dp["poly_n"]) <= 8
        for i, (_a, b, _v) in enumerate(dp["poly_n"]):
            cv[24 + i] = b
    cvec_arr = np.broadcast_to(cv, (P, 32)).copy()

    in_maps = []
    for c in range(cfg.ncores):
        quad_base, lane_edge = packs[c]
        nq = len(quad_base)
        # quad q -> (p = q % 128, kq = q // 128); offsets are PAIR-row ids
        colw = np.zeros((P, cfg.KQ), np.int32)
        ql = np.arange(nq)
        colw[ql % P, ql // P] = quad_base // 2
        # eat packed [72, K, 128]: pseudo-slot (p, k=kq*QG+j) lane l
        eatp = np.zeros((9 * G_LANES, cfg.K, P), np.float32)
        le = lane_edge.reshape(nq, QG, G_LANES)
        valid = le >= 0
        ea_l = np.zeros((nq, QG, G_LANES, ED), np.float32)
        ea_l[valid] = edge_attr[le[valid]]
        pp = (ql % P)
        kq = (ql // P)
        for j in range(QG):
            kk = kq * QG + j
            for l in range(G_LANES):
                eatp[9 * l:9 * l + ED, kk, pp] = ea_l[:, j, l, :].T
                eatp[9 * l + ED, kk, pp] = valid[:, j, l].astype(np.float32)
        in_maps.append({
            "xt": np.ascontiguousarray(xt_all[:, c * cfg.ns:(c + 1) * cfg.ns]),
            "wlin": wlin_aug,
            "eat": _tobf16(eatp),
            "wbd": wbd,
            "colv": colw,
            "cvec": cvec_arr,
        })
    return in_maps


def run(inputs, trace=False):
    from concourse.bass_utils import run_bass_kernel_spmd

    _patch_compiler_flags()
    col = np.asarray(inputs["col"], np.int32)
    n_nodes = inputs["x"].shape[0]
    e_edges = col.shape[0]

    ns = -(-(-(-n_nodes // NCORES)) // P) * P
    owner = np.minimum(col // ns, NCORES - 1)
    packs = []
    nq_max = 1
    for c in range(NCORES):
        eidx = np.nonzero(owner == c)[0]
        qb, le = pack_core(col[eidx] - c * ns, eidx, ns)
        packs.append((qb, le))
        nq_max = max(nq_max, len(qb))
    cfg = Cfg(n_nodes, e_edges, NCORES, nq_max)

    dp = derive_params(inputs)
    assert dp["poly"] is not None, "poly fit failed"
    nc = build_graph(cfg, dp)
    _split_multi_waits(nc)
    in_maps = host_prep(inputs, cfg, dp, packs)
    res = run_bass_kernel_spmd(nc, in_maps, list(range(cfg.ncores)), trace=trace)
    full = np.empty((e_edges, 2 * C), np.float32)
    for c in range(cfg.ncores):
        quad_base, lane_edge = packs[c]
        o = np.asarray(res.results[c]["out"]).astype(np.float32)
        # [P, KQ, QG, G_LANES, 2C] -> quad q=(p + 128kq), group j, lane l
        o = o.reshape(P, cfg.KQ, QG, G_LANES, 2 * C).transpose(1, 0, 2, 3, 4)
        o = o.reshape(cfg.KQ * P * QG * G_LANES, 2 * C)
        valid = lane_edge >= 0
        full[lane_edge[valid]] = o[:len(lane_edge)][valid]
    return full, res


def kernel(**inputs):
    full, _ = run(inputs, trace=False)
    return full


# revision 31
# speedup vs baseline: 1.1488x; 1.0722x over previous
"""AttentionGCNConv edge kernel for 8 Trainium2 NeuronCores (v2).

Strategy (edge-sharded SPMD, no cross-core communication):
  * Edges bucketed by destination-node range (ns nodes per core) so every
    gather is core-local, then sorted by col and packed into 8-lane SLOTS:
    all 8 lanes of a slot share one destination node, so one 72-byte
    indirect-DMA descriptor serves 8 edges.  The Q7 SWDGE fixed cost
    (994 ns/instruction) is amortized 8x vs. the per-edge baseline.
  * Node phase computes h = x@W+b, the exact per-scalar MLP f on h,
    gn = exp(f(h)), G = sum_c gn, and packs bf16 table rows
    {P_left = repeat2(h_lo)*gn [16], P_right = repeat2(h_hi) [16], G}.
  * Edge phase computes scores = edge_attr@W_edge+b_edge with BLOCK-DIAGONAL
    packed matmuls: 8 lanes x 9 contraction rows = 72-row lhsT so one matmul
    produces 1024 edge-scores (vs 128 in the naive per-chunk form).
    exp(f(score)) via a host-fitted product-of-quadratics polynomial
    (ACT Square + DVE fused ops), exact 16-branch fallback.
  * Combine: D = G + sum_c exp(f(score)), R = 1/D, out = {P_left*R,
    P_right*R*gea} per lane, written bf16 and widened to f32 on host.
"""
import numpy as np


# ---------------------------------------------------------------------------
# problem constants (hardcoded per the task statement)
# ---------------------------------------------------------------------------
N_NODES = 100000
E_EDGES = 1000000
IN_C = 64
C = 16          # OUT_C
ED = 8          # EDGE_D
NCORES = 8
P = 128
G_LANES = 8     # edges per lane-group (all share one destination node)
QG = 2          # lane-groups per pair-slot (2 consecutive nodes, one 144B desc)
TW = 36         # table row width in bf16 (72B): 16 P_left, 16 P_right, G, pad
KCQ = 8         # pair-columns processed per pipeline chunk (= 16 pseudo-cols)


class Cfg:
    def __init__(self, n_nodes, e_edges, ncores, nq_max, in_c=IN_C):
        self.ncores = ncores
        self.in_c = in_c
        # node shard: multiple of 128
        ns = -(-n_nodes // ncores)
        self.ns = -(-ns // P) * P
        self.nchunks = self.ns // P
        self.n_nodes = n_nodes
        self.e_edges = e_edges
        # quad grid: KQ columns of 128 quads, KQ multiple of KCQ
        kq = -(-nq_max // P)
        self.KQ = -(-kq // KCQ) * KCQ
        self.K = self.KQ * QG                 # pseudo-slot columns
        self.nslots = self.K * P
        self.lanes = self.nslots * G_LANES    # padded edge-lane count


# ---------------------------------------------------------------------------
# host-side derived parameters
# ---------------------------------------------------------------------------
def _f_scalar(s, w1, b1, w2, b2):
    z = s[..., None] * w1 + b1
    return (np.maximum(z, 0.0) * w2).sum(-1) + b2[0]


def fit_poly_factors(w1, b1, w2, b2, lo, hi):
    """Fit exp(f(s)) on [lo, hi] by a polynomial that factors into real
    quadratics (s+u)^2 + v scaled by alpha = c_lead^(1/nf).  Returns
    (factors, max_rel_err) or None if no degree works."""
    grid = np.linspace(lo, hi, 8192)
    target = np.exp(_f_scalar(grid, w1, b1, w2, b2))
    for deg in (6, 8, 10, 12, 14):
        ch = np.polynomial.chebyshev.Chebyshev.fit(grid, target, deg)
        p = ch.convert(kind=np.polynomial.Polynomial)
        c_lead = p.coef[-1]
        if c_lead <= 0:
            continue
        roots = p.roots()
        creal = sorted([r.real for r in roots if abs(r.imag) < 1e-12])
        ccplx = [r for r in roots if r.imag > 1e-12]
        if len(creal) % 2 != 0:
            continue
        quads = [(-r.real, r.imag ** 2) for r in ccplx]
        for i in range(0, len(creal), 2):
            r1, r2 = creal[i], creal[i + 1]
            quads.append((-(r1 + r2) / 2.0, -(((r1 - r2) / 2.0) ** 2)))
        nf = len(quads)
        alpha = c_lead ** (1.0 / nf)
        sa = float(np.sqrt(alpha))
        facs = [(sa, sa * u, alpha * v) for (u, v) in quads]
        acc = np.ones_like(grid)
        for (a, b, v) in facs:
            acc = acc * ((a * grid + b) ** 2 + v)
        rel = np.abs(acc - target) / np.abs(target)
        if rel.max() < 4.5e-3:
            return facs, float(rel.max())
    return None


def derive_params(inputs):
    w1 = np.asarray(inputs["w1"], np.float64)
    b1 = np.asarray(inputs["b1"], np.float64)
    w2 = np.asarray(inputs["w2"], np.float64)
    b2 = np.asarray(inputs["b2"], np.float64)
    W_edge = np.asarray(inputs["W_edge"], np.float64)
    b_edge = np.asarray(inputs["b_edge"], np.float64)

    aw = w1 * np.abs(w2)
    cw = b1 * np.abs(w2)
    sg = np.sign(w2)

    sigma_c = np.sqrt((W_edge ** 2).sum(0))
    lo = float((b_edge - 6.5 * sigma_c).min())
    hi = float((b_edge + 6.5 * sigma_c).max())
    fit = fit_poly_factors(w1, b1, w2, b2, lo, hi)

    # node-side poly over the EXACT h range (computed on host for the fit
    # range only; h itself is computed on device)
    x = np.asarray(inputs["x"], np.float64)
    W_lin = np.asarray(inputs["W_lin"], np.float64)
    b_lin = np.asarray(inputs["b_lin"], np.float64)
    h = x @ W_lin + b_lin
    mg = 1e-3 * (h.max() - h.min())
    fit_n = fit_poly_factors(w1, b1, w2, b2, float(h.min()) - mg,
                             float(h.max()) + mg)
    return {
        "aw": aw, "cw": cw, "sg": sg, "b2": float(b2[0]),
        "lo": lo, "hi": hi,
        "poly": None if fit is None else fit[0],
        "poly_err": None if fit is None else fit[1],
        "poly_n": None if fit_n is None else fit_n[0],
    }


# ---------------------------------------------------------------------------
# graph builder (SPMD, one graph for all cores)
# ---------------------------------------------------------------------------
def build_graph(cfg, dp):
    from concourse import bass, mybir
    import concourse.tile as tile

    f32 = mybir.dt.float32
    bf16 = mybir.dt.bfloat16
    i32 = mybir.dt.int32
    ALU = mybir.AluOpType
    ACTF = mybir.ActivationFunctionType

    nc = bass.Bass()
    xt = nc.declare_dram_parameter("xt", [cfg.in_c + 1, cfg.ns], bf16, isOutput=False)
    wlin = nc.declare_dram_parameter("wlin", [cfg.in_c + 1, C], bf16, isOutput=False)
    # packed edge attrs: [72, K, 128]: row 9l+d = attr d of lane l
    eat = nc.declare_dram_parameter("eat", [9 * G_LANES, cfg.K, P], bf16, isOutput=False)
    # block-diagonal W_edge: [72, 128]: rows 9l+d, cols 16l+c
    wbd = nc.declare_dram_parameter("wbd", [9 * G_LANES, P], bf16, isOutput=False)
    colv = nc.declare_dram_parameter("colv", [P, cfg.KQ], i32, isOutput=False)
    cvec = nc.declare_dram_parameter("cvec", [P, 32], f32, isOutput=False)
    out_e = nc.declare_dram_parameter(
        "out", [P, cfg.K * G_LANES * 2 * C], bf16, isOutput=True)
    # cvec columns: 0..15 = cw_k, 16 = b2, 17.. = poly bias b_i

    # Node-PAIR rows (72 bf16 = 144B) so the indirect-DMA descriptor length
    # (coef x 4B, an f32-element assumption in the DGE) covers exactly one
    # quad (288B) per offset.  +QG pad rows for quads based near ns.
    tableL = nc.dram_tensor("tableL", [(cfg.ns + 2 * QG) // 2, 2 * TW], bf16)

    NCH = cfg.nchunks
    NPOLY = 0 if dp["poly"] is None else len(dp["poly"])

    with tile.TileContext(nc) as tc, nc.allow_low_precision(
            reason="bf16 poly chain & recip; within the 2e-2 rel-err budget"):
        with tc.tile_pool(name="const", bufs=1) as constp:
            wlin_sb = constp.tile([cfg.in_c + 1, C], bf16)
            nc.sync.dma_start(out=wlin_sb[:], in_=wlin[:])
            wbd_sb = constp.tile([9 * G_LANES, P], bf16)
            nc.sync.dma_start(out=wbd_sb[:], in_=wbd[:])
            cvec_sb = constp.tile([P, 32], f32)
            nc.sync.dma_start(out=cvec_sb[:], in_=cvec[:])
            colv_sb = constp.tile([P, cfg.KQ], i32)
            nc.sync.dma_start(out=colv_sb[:], in_=colv[:])

            # ---------------- phase 0: node table ----------------
            with (
                tc.tile_pool(name="node_sb", bufs=1) as np_sb,
                tc.tile_pool(name="node_ps", bufs=2, space="PSUM") as np_ps,
            ):
                xt_sb = np_sb.tile([cfg.in_c + 1, cfg.ns], bf16)
                nc.sync.dma_start(out=xt_sb[:], in_=xt[:])

                h_wide = np_sb.tile([P, NCH * C], f32)
                for g0 in range(0, NCH, 32):
                    g1 = min(g0 + 32, NCH)
                    pst = np_ps.tile([P, 512], f32, tag="np_ps")
                    for j in range(g0, g1):
                        nc.tensor.matmul(
                            out=pst[:, (j - g0) * C:(j - g0 + 1) * C],
                            lhsT=xt_sb[:, j * P:(j + 1) * P],
                            rhs=wlin_sb[:],
                            start=True, stop=True,
                        )
                    nc.scalar.copy(
                        out=h_wide[:, g0 * C:g1 * C],
                        in_=pst[:, :(g1 - g0) * C],
                    )

                # gn = exp(f(h)) via the node-range poly (exact 16-branch
                # fallback if the fit failed)
                FW = NCH * C
                gn_wide = np_sb.tile([P, FW], f32)
                tmp = np_sb.tile([P, FW], f32)
                acc_a = np_sb.tile([P, FW], f32)
                acc_b = np_sb.tile([P, FW], f32)
                if dp["poly_n"] is not None:
                    npn = len(dp["poly_n"])
                    for i, (a, b, v) in enumerate(dp["poly_n"]):
                        nc.scalar.activation(
                            out=tmp[:], in_=h_wide[:], func=ACTF.Square,
                            bias=cvec_sb[:, 24 + i:25 + i], scale=float(a),
                        )
                        if i == 0:
                            nc.vector.tensor_scalar(
                                out=acc_a[:], in0=tmp[:], scalar1=float(v),
                                scalar2=None, op0=ALU.add,
                            )
                        else:
                            src, dst = (acc_a, acc_b) if i % 2 == 1 else (acc_b, acc_a)
                            nc.vector.scalar_tensor_tensor(
                                out=(gn_wide[:] if i == npn - 1 else dst[:]),
                                in0=tmp[:], scalar=float(v), in1=src[:],
                                op0=ALU.add, op1=ALU.mult,
                            )
                else:
                    for k in range(C):
                        nc.scalar.activation(
                            out=tmp[:], in_=h_wide[:], func=ACTF.Relu,
                            bias=cvec_sb[:, k:k + 1], scale=float(dp["aw"][k]),
                        )
                        src, dst = (acc_a, acc_b) if k % 2 == 1 else (acc_b, acc_a)
                        if k == 0:
                            nc.vector.tensor_scalar(
                                out=acc_a[:], in0=tmp[:],
                                scalar1=float(dp["sg"][k]), scalar2=None,
                                op0=ALU.mult,
                            )
                        else:
                            nc.vector.scalar_tensor_tensor(
                                out=dst[:], in0=tmp[:], scalar=float(dp["sg"][k]),
                                in1=src[:], op0=ALU.mult, op1=ALU.add,
                            )
                    acc_fin = acc_a if C % 2 == 1 else acc_b
                    nc.scalar.activation(
                        out=gn_wide[:], in_=acc_fin[:], func=ACTF.Exp,
                        bias=cvec_sb[:, 16:17], scale=1.0,
                    )
                g_wide = np_sb.tile([P, NCH], f32)
                nc.vector.tensor_reduce(
                    out=g_wide[:], in_=gn_wide[:].rearrange("p (j c) -> p j c", c=C),
                    axis=mybir.AxisListType.X, op=ALU.add,
                )

                # table rows (bf16, 72B): row n = j*128 + p
                rowbuf = np_sb.tile([P, NCH, TW], bf16)
                hv = h_wide[:].rearrange("p (j c) -> p j c", c=C)
                # P_left[c] = h[c//2] * gn[c]
                nc.vector.tensor_tensor(
                    out=rowbuf[:, :, 0:C].rearrange("p j (h two) -> p j h two", two=2),
                    in0=hv[:, :, 0:C // 2].unsqueeze(-1).to_broadcast(
                        [P, NCH, C // 2, 2]),
                    in1=gn_wide[:].rearrange(
                        "p (j h two) -> p j h two", h=C // 2, two=2),
                    op=ALU.mult,
                )
                # P_right[c] = h[8 + c//2]
                nc.vector.tensor_copy(
                    out=rowbuf[:, :, C:2 * C].rearrange(
                        "p j (h two) -> p j h two", two=2),
                    in_=hv[:, :, C // 2:C].unsqueeze(-1).to_broadcast(
                        [P, NCH, C // 2, 2]),
                )
                nc.vector.tensor_copy(
                    out=rowbuf[:, :, 2 * C:2 * C + 1],
                    in_=g_wide[:].unsqueeze(-1),
                )
                nc.vector.memset(rowbuf[:, :, 2 * C + 1:TW], 0.0)
                tl_rows = tableL[:].rearrange("r (h c) -> (r h) c", h=2)
                # node n = p*NCH + j lives at (partition p, slot j): each
                # partition writes one contiguous 7.2KB block (128 big
                # descriptors instead of 12800 x 72B)
                nc.sync.dma_start(
                    out=tl_rows[0:cfg.ns].rearrange("(p j) c -> p j c", j=NCH),
                    in_=rowbuf[:],
                )
                padrow = np_sb.tile([2 * QG, TW], bf16)
                nc.vector.memset(padrow[:], 1.0)
                nc.sync.dma_start(
                    out=tl_rows[cfg.ns:cfg.ns + 2 * QG], in_=padrow[:])

            # ----- fused edge pipeline: chunks of KCQ quad-columns -----
            NCHK = cfg.KQ // KCQ
            KC = KCQ * QG         # pseudo-slot columns per chunk
            FWC = KC * P          # lane-scores per chunk
            with (
                tc.tile_pool(name="ek", bufs=1) as ek,
                tc.tile_pool(name="esb", bufs=3) as esb,
                tc.tile_pool(name="eps", bufs=2, space="PSUM") as eps,
                tc.tile_pool(name="gsb", bufs=3) as gsb,
            ):
                gea_full = ek.tile([P, cfg.K * P], bf16)

                for t in range(NCHK):
                    kq0 = t * KCQ
                    k0 = t * KC
                    # gather first so the Q7 stays busy: one 288B descriptor
                    # per quad covers 4 consecutive table rows
                    grow = gsb.tile([P, KC, TW], bf16, tag="grow")
                    for kk in range(KCQ):
                        # dest MUST be a flat 2D AP: a 3D dest splits into
                        # one descriptor per middle-dim element, consuming
                        # extra (garbage) offsets
                        nc.gpsimd.indirect_dma_start(
                            out=grow[:, kk * QG:(kk + 1) * QG, :].rearrange(
                                "p q d -> p (q d)"),
                            out_offset=None,
                            in_=tableL[:],
                            in_offset=bass.IndirectOffsetOnAxis(
                                ap=colv_sb[:, kq0 + kk:kq0 + kk + 1], axis=0),
                        )

                    # packed ea matmuls: one per pseudo-column, into a
                    # 2-bank psum tile read directly by the poly ACTs
                    eat_t = esb.tile([9 * G_LANES, KC, P], bf16, tag="eat_t")
                    nc.sync.dma_start(out=eat_t[:], in_=eat[:, k0:k0 + KC, :])
                    pse = eps.tile([P, FWC], f32, tag="ea_ps")
                    for kk in range(KC):
                        nc.tensor.matmul(
                            out=pse[:, kk * P:(kk + 1) * P],
                            lhsT=eat_t[:, kk, :],
                            rhs=wbd_sb[:],
                            start=True, stop=True,
                        )

                    gea_t = gea_full[:, k0 * P:(k0 + KC) * P]
                    sq = esb.tile([P, FWC], bf16, tag="sq")
                    qa = esb.tile([P, FWC], bf16, tag="qa")
                    qb = esb.tile([P, FWC], bf16, tag="qb")
                    for i, (a, b, v) in enumerate(dp["poly"]):
                        nc.scalar.activation(
                            out=sq[:], in_=pse[:], func=ACTF.Square,
                            bias=cvec_sb[:, 17 + i:18 + i], scale=float(a),
                        )
                        if i == 0:
                            nc.vector.tensor_scalar(
                                out=qa[:], in0=sq[:], scalar1=float(v),
                                scalar2=None, op0=ALU.add,
                            )
                        else:
                            src, dst = (qa, qb) if i % 2 == 1 else (qb, qa)
                            last = i == NPOLY - 1
                            nc.vector.scalar_tensor_tensor(
                                out=(gea_t if last else dst[:]),
                                in0=sq[:], scalar=float(v), in1=src[:],
                                op0=ALU.add, op1=ALU.mult,
                            )

                    # easum per lane, D = G + easum, R = 1/D (bf16)
                    easum = gsb.tile([P, KC * G_LANES], f32, tag="easum")
                    nc.vector.tensor_reduce(
                        out=easum[:],
                        in_=gea_t.rearrange("p (k l c) -> p (k l) c", l=G_LANES, c=C),
                        axis=mybir.AxisListType.X, op=ALU.add,
                    )
                    d_t = gsb.tile([P, KC, G_LANES], f32, tag="d_t")
                    nc.vector.tensor_tensor(
                        out=d_t[:],
                        in0=grow[:, :, 2 * C:2 * C + 1].to_broadcast(
                            [P, KC, G_LANES]),
                        in1=easum[:].rearrange("p (k l) -> p k l", l=G_LANES),
                        op=ALU.add,
                    )
                    r_t = gsb.tile([P, KC, G_LANES], bf16, tag="r_t")
                    nc.vector.reciprocal(out=r_t[:], in_=d_t[:])

                    out_t = gsb.tile([P, KC, G_LANES, 2 * C], bf16, tag="out_t")
                    # left: P_left * R
                    nc.vector.tensor_tensor(
                        out=out_t[:, :, :, 0:C],
                        in0=grow[:, :, 0:C].unsqueeze(2).to_broadcast(
                            [P, KC, G_LANES, C]),
                        in1=r_t[:].unsqueeze(-1).to_broadcast(
                            [P, KC, G_LANES, C]),
                        op=ALU.mult,
                    )
                    # right: (gea * R) * P_right
                    wr = gsb.tile([P, KC, G_LANES, C], bf16, tag="wr")
                    nc.vector.tensor_tensor(
                        out=wr[:],
                        in0=gea_t.rearrange("p (k l c) -> p k l c", l=G_LANES, c=C),
                        in1=r_t[:].unsqueeze(-1).to_broadcast(
                            [P, KC, G_LANES, C]),
                        op=ALU.mult,
                    )
                    nc.vector.tensor_tensor(
                        out=out_t[:, :, :, C:2 * C],
                        in0=wr[:],
                        in1=grow[:, :, C:2 * C].unsqueeze(2).to_broadcast(
                            [P, KC, G_LANES, C]),
                        op=ALU.mult,
                    )
                    nc.sync.dma_start(
                        out=out_e[:].rearrange(
                            "p (k l c) -> p k l c", l=G_LANES, c=2 * C)[
                            :, k0:k0 + KC, :, :],
                        in_=out_t[:],
                    )
    return nc


# ---------------------------------------------------------------------------
# walrus single-wait post-pass
# ---------------------------------------------------------------------------
def _split_multi_waits(nc):
    """This walrus build supports at most one sem-wait per instruction;
    hoist extra waits onto single-wait NoOps inserted just before."""
    from concourse import mybir
    ctr = [0]
    for f in nc.m.functions:
        for bb in f.blocks:
            il = bb.instructions
            new = []
            for inst in il:
                si = inst.sync_info
                waits = list(si.on_wait) if (si is not None and si.on_wait) else []
                if len(waits) > 1:
                    for w in waits[:-1]:
                        ctr[0] += 1
                        nop = mybir.InstNoOp(
                            name=f"splitw-{ctr[0]}", ins=[], outs=[])
                        nop.engine = inst.engine
                        nop.sync_info = mybir.SyncInfo(on_wait=[w], on_update=[])
                        new.append(nop)
                    si.on_wait = [waits[-1]]
                new.append(inst)
            il[:] = new
    return ctr[0]


def _patch_compiler_flags():
    """Enable the vector_dynamic_offsets DGE level (needed by the indirect
    gather); the default flag bundle disables it."""
    from concourse.compiler_utils import get_compiler_flags, set_compiler_flags
    flags = list(get_compiler_flags())
    if not flags:
        return
    out = []
    i = 0
    while i < len(flags):
        if flags[i] == "--internal-disable-dge-levels":
            i += 1
            while i < len(flags) and not flags[i].startswith("-"):
                i += 1
            continue
        out.append(flags[i])
        i += 1
    if "--internal-enable-dge-levels" in out:
        j = out.index("--internal-enable-dge-levels")
        if "vector_dynamic_offsets" not in out:
            out.insert(j + 1, "vector_dynamic_offsets")
    set_compiler_flags(out)


# ---------------------------------------------------------------------------
# host prep + entry
# ---------------------------------------------------------------------------
def _tobf16(x):
    import ml_dtypes
    return np.asarray(x, dtype=ml_dtypes.bfloat16)


def pack_core(col_loc, eidx, ns):
    """Quad-pack one core's edges: sort by local col; each node needs
    ceil(cnt/8) lane-groups; greedily cover lane-groups with quads of 4
    consecutive nodes (one 288B gather descriptor per quad).

    Returns (quad_base[int32 nq], lane_edge[int64 nq*4*8, global edge id
    or -1])."""
    order = np.argsort(col_loc, kind="stable")
    cs = col_loc[order]
    es = eidx[order]
    ne = len(cs)
    if ne == 0:
        return np.zeros(0, np.int32), np.zeros(0, np.int64)
    cnt = np.bincount(cs, minlength=ns)
    starts = np.concatenate([[0], np.cumsum(cnt)[:-1]])
    g = -(-cnt // G_LANES)
    rem = g.copy()
    nz = np.nonzero(rem)[0]
    quad_base = []
    consumed = []
    ptr = 0          # index into nz of first node with rem > 0
    n_nz = len(nz)
    while ptr < n_nz:
        n = nz[ptr]
        if rem[n] == 0:
            ptr += 1
            continue
        b = n & ~1          # even base: table rows are node pairs
        take = 0
        for j in range(QG):
            m = b + j
            if m < ns and rem[m] > 0:
                rem[m] -= 1
                take |= 1 << j
        quad_base.append(b)
        consumed.append(take)
    nq = len(quad_base)
    lane_edge = np.full((nq, QG, G_LANES), -1, np.int64)
    cursor = np.zeros(ns, np.int64)
    for q in range(nq):
        b = quad_base[q]
        tk = consumed[q]
        for j in range(QG):
            if tk & (1 << j):
                m = b + j
                s0 = starts[m] + cursor[m]
                k = min(G_LANES, cnt[m] - cursor[m])
                lane_edge[q, j, :k] = es[s0:s0 + k]
                cursor[m] += k
    return np.asarray(quad_base, np.int32), lane_edge.reshape(-1)


def host_prep(inputs, cfg, dp, packs):
    edge_attr = np.asarray(inputs["edge_attr"], np.float32)
    x = np.asarray(inputs["x"], np.float32)
    W_lin = np.asarray(inputs["W_lin"], np.float32)
    b_lin = np.asarray(inputs["b_lin"], np.float32)
    W_edge = np.asarray(inputs["W_edge"], np.float32)
    b_edge = np.asarray(inputs["b_edge"], np.float32)

    n = cfg.n_nodes
    nt_all = cfg.ns * cfg.ncores
    xt_all = np.zeros((cfg.in_c + 1, nt_all), np.float32)
    xt_all[:cfg.in_c, :n] = x.T
    xt_all[cfg.in_c, :] = 1.0
    xt_all = _tobf16(xt_all)
    wlin_aug = _tobf16(np.concatenate([W_lin, b_lin[None, :]], 0))

    # block-diagonal W_edge [72, 128]
    wbd = np.zeros((9 * G_LANES, P), np.float32)
    for l in range(G_LANES):
        wbd[9 * l:9 * l + ED, C * l:C * (l + 1)] = W_edge
        wbd[9 * l + ED, C * l:C * (l + 1)] = b_edge
    wbd = _tobf16(wbd)

    cv = np.zeros(32, np.float32)
    cv[:C] = dp["cw"]
    cv[16] = dp["b2"]
    for i, (_a, b, _v) in enumerate(dp["poly"]):
        cv[17 + i] = b
    if dp["poly_n"] is not None:
        assert len(dp["poly_n"]) <= 8
        for i, (_a, b, _v) in enumerate(dp["poly_n"]):
            cv[24 + i] = b
    cvec_arr = np.broadcast_to(cv, (P, 32)).copy()

    in_maps = []
    for c in range(cfg.ncores):
        quad_base, lane_edge = packs[c]
        nq = len(quad_base)
        # quad q -> (p = q % 128, kq = q // 128); offsets are PAIR-row ids
        colw = np.zeros((P, cfg.KQ), np.int32)
        ql = np.arange(nq)
        colw[ql % P, ql // P] = quad_base // 2
        # eat packed [72, K, 128]: pseudo-slot (p, k=kq*QG+j) lane l
        eatp = np.zeros((9 * G_LANES, cfg.K, P), np.float32)
        le = lane_edge.reshape(nq, QG, G_LANES)
        valid = le >= 0
        ea_l = np.zeros((nq, QG, G_LANES, ED), np.float32)
        ea_l[valid] = edge_attr[le[valid]]
        pp = (ql % P)
        kq = (ql // P)
        for j in range(QG):
            kk = kq * QG + j
            for l in range(G_LANES):
                eatp[9 * l:9 * l + ED, kk, pp] = ea_l[:, j, l, :].T
                eatp[9 * l + ED, kk, pp] = valid[:, j, l].astype(np.float32)
        # device matmul chunk j puts xt column j*128+p on partition p; the
        # table write needs node p*NCH+j there, so permute columns
        dev = np.arange(cfg.ns)
        perm = (dev % P) * cfg.nchunks + dev // P
        in_maps.append({
            "xt": np.ascontiguousarray(xt_all[:, c * cfg.ns + perm]),
            "wlin": wlin_aug,
            "eat": _tobf16(eatp),
            "wbd": wbd,
            "colv": colw,
            "cvec": cvec_arr,
        })
    return in_maps


def run(inputs, trace=False):
    from concourse.bass_utils import run_bass_kernel_spmd

    _patch_compiler_flags()
    col = np.asarray(inputs["col"], np.int32)
    n_nodes = inputs["x"].shape[0]
    e_edges = col.shape[0]

    ns = -(-(-(-n_nodes // NCORES)) // P) * P
    owner = np.minimum(col // ns, NCORES - 1)
    packs = []
    nq_max = 1
    for c in range(NCORES):
        eidx = np.nonzero(owner == c)[0]
        qb, le = pack_core(col[eidx] - c * ns, eidx, ns)
        packs.append((qb, le))
        nq_max = max(nq_max, len(qb))
    cfg = Cfg(n_nodes, e_edges, NCORES, nq_max)

    dp = derive_params(inputs)
    assert dp["poly"] is not None, "poly fit failed"
    nc = build_graph(cfg, dp)
    _split_multi_waits(nc)
    in_maps = host_prep(inputs, cfg, dp, packs)
    res = run_bass_kernel_spmd(nc, in_maps, list(range(cfg.ncores)), trace=trace)
    full = np.empty((e_edges, 2 * C), np.float32)
    for c in range(cfg.ncores):
        quad_base, lane_edge = packs[c]
        o = np.asarray(res.results[c]["out"]).astype(np.float32)
        # [P, KQ, QG, G_LANES, 2C] -> quad q=(p + 128kq), group j, lane l
        o = o.reshape(P, cfg.KQ, QG, G_LANES, 2 * C).transpose(1, 0, 2, 3, 4)
        o = o.reshape(cfg.KQ * P * QG * G_LANES, 2 * C)
        valid = lane_edge >= 0
        full[lane_edge[valid]] = o[:len(lane_edge)][valid]
    return full, res


def kernel(**inputs):
    full, _ = run(inputs, trace=False)
    return full
